# revision 24
# baseline (speedup 1.0000x reference)
"""Child-Sum Tree-LSTM (reference.py nn_ChildSumTreeLSTM) on 8 Trainium2
NeuronCores via Bass/Tile, SPMD.

Strategy: everything transposed (features on SBUF partitions, nodes on the
free dimension). Each core owns a contiguous slice of levels 7..8; since
children of a node are contiguous, the leaves->level-7 recursion is fully
core-local (no collectives). Levels 6..0 (5461 nodes) are finished on the
host in numpy during the gather step.

The activation engine is the binding resource (1 elem/cycle/lane plus a
~352-cycle fixed cost per ACTIVATE), so gates are produced from [P, 2048]
four-bank PSUM tiles and activated in N=2048 calls (N=4096 for the tanh(c)
that needs no bias). fp16 replaces bf16 at identical engine throughput for
~8x less quantization noise. Leaf groups are software-pipelined one group
ahead of the level-7 forget-gate matmuls so the TensorEngine never stalls
on the activation chain it just fed (keeps the PE HAM clock at 2.4 GHz).
Level-7 iou runs in halves with per-half output DMA so the final DMA
overlaps compute. Weights/x for later phases load on the Vector engine's
DMA queue in parallel with the Sync queue's leaf x stream.
"""
import sys
sys.path.insert(0, '/opt/trn_rl_repo')
import numpy as np
import concourse.bacc as bacc
import concourse.mybir as mybir
from concourse.tile import TileContext
from concourse.alu_op_type import AluOpType

F32 = mybir.dt.float32
F16 = mybir.dt.float16
AFT = mybir.ActivationFunctionType
P = 128
NCORES = 8
BR = 4
D = 8
CUT = 7

NLOC = {7: 2048, 8: 8192}
LOFF = {7: 0, 8: 2048}
TOTAL_ROWS = 10240
GROUPS = ((0, 2048), (2048, 2048), (4096, 2048), (6144, 2048))
NG = len(GROUPS)

# gate emission order: (wx block index, act fn, name); wx free layout is
# [i(256) | o(256) | u(256) | f(256)], bias cols [i0,i1,o0,o1,u0,u1,f0,f1]
GATES = ((0, AFT.Sigmoid, "i"), (2, AFT.Tanh, "u"), (1, AFT.Sigmoid, "o"))


def build_program():
    nc = bacc.Bacc("TRN2", target_bir_lowering=False, debug=False,
                   num_devices=NCORES)
    xT = nc.dram_tensor("xT", [2, P, TOTAL_ROWS], F16, kind="ExternalInput")
    wx = nc.dram_tensor("wx", [2, P, 1024], F16, kind="ExternalInput")
    wh = nc.dram_tensor("wh", [2, P, 1024], F16, kind="ExternalInput")
    bias = nc.dram_tensor("bias", [P, 8], F32, kind="ExternalInput")
    out_h = nc.dram_tensor("out_h", [2, P, NLOC[7]], F16, kind="ExternalOutput")
    out_c = nc.dram_tensor("out_c", [2, P, NLOC[7]], F16, kind="ExternalOutput")

    with TileContext(nc) as tc:
        with tc.tile_pool(name="const", bufs=1) as constp, \
             tc.tile_pool(name="xin", bufs=2) as xin, \
             tc.tile_pool(name="state", bufs=1) as statep, \
             tc.tile_pool(name="leafg", bufs=2) as leafg, \
             tc.tile_pool(name="work", bufs=2) as work, \
             tc.tile_pool(name="psum", bufs=2, space="PSUM") as psum:

            wxt = constp.tile([P, 2, 1024], F16)
            bt = constp.tile([P, 8], F32)
            wht = constp.tile([P, 2, 1024], F16)
            nc.sync.dma_start(wxt[:], wx[:].rearrange("a p n -> p a n"))
            nc.sync.dma_start(bt[:], bias[:])

            def load_x(l, c0, S, tag, bufs=2, split=1):
                t = xin.tile([P, 2, S], F16, tag=tag, bufs=bufs, name=tag)
                w = S // split
                for j in range(split):
                    lo = LOFF[l] + c0 + j * w
                    nc.sync.dma_start(
                        t[:, :, j * w:(j + 1) * w],
                        xT[:, :, lo:lo + w].rearrange("a p n -> p a n"))
                return t

            # group 0's x is split into 512-column pieces so the first
            # matmul starts as soon as wx + one piece have landed; x7/wht
            # transfers queue up behind it instead of competing for HBM
            nc.scalar.dma_start(wht[:], wh[:].rearrange("a p n -> p a n"))
            xt_g = [load_x(8, 0, GROUPS[0][1], tag="xl0", bufs=1, split=2)]
            x7 = load_x(7, 0, NLOC[7], tag="x7", bufs=1)

            # persistent level-7 state
            hs7 = statep.tile([P, 2, NLOC[7]], F16, name="hs7")
            fcs7 = statep.tile([P, 2, NLOC[7]], F16, name="fcs7")
            h7 = statep.tile([P, 2, NLOC[7]], F16, name="h7")
            c7 = statep.tile([P, 2, NLOC[7]], F16, name="c7")

            def fill_iou(ps, xt, S, gi, ft, hs=None):
                """Fill [P, S] psum AP for gate-block gi, feature-tile ft."""
                sl = slice((gi * 2 + ft) * P, (gi * 2 + ft + 1) * P)
                for q in range(0, S, 512):
                    w = min(512, S - q)
                    dst = ps[:, q:q + w]
                    nc.tensor.matmul(dst, wxt[:, 0, sl], xt[:, 0, q:q + w],
                                     start=True, stop=False)
                    nc.tensor.matmul(dst, wxt[:, 1, sl], xt[:, 1, q:q + w],
                                     start=False, stop=hs is None)
                    if hs is not None:
                        nc.tensor.matmul(dst, wht[:, 0, sl], hs[:, 0, q:q + w],
                                         start=False, stop=False)
                        nc.tensor.matmul(dst, wht[:, 1, sl], hs[:, 1, q:q + w],
                                         start=False, stop=True)

            def f_psum(ch_h, xp, S, ft):
                """[P, 4*S] forget-gate pre-activation psum for S parents:
                W_fh @ child_h + (W_fx @ x_parent) broadcast over children."""
                nch = BR * S
                ps = psum.tile([P, 2048], F32, tag="PS", bufs=2, name="psf")
                sl = slice(768 + ft * P, 768 + (ft + 1) * P)
                for q in range(0, nch, 512):
                    w = min(512, nch - q)
                    dst = ps[:, q:q + w]
                    nc.tensor.matmul(dst, wht[:, 0, sl], ch_h[:, 0, q:q + w],
                                     start=True, stop=False)
                    nc.tensor.matmul(dst, wht[:, 1, sl], ch_h[:, 1, q:q + w],
                                     start=False, stop=False)
                    plo, pw = q // BR, w // BR
                    for kt in range(2):
                        rhs = xp[:, kt, plo:plo + pw] \
                            .rearrange("p (n b) -> p n b", b=1) \
                            .broadcast_to([P, pw, BR])
                        nc.tensor.matmul(
                            dst.rearrange("p (n b) -> p n b", b=BR),
                            wxt[:, kt, sl], rhs, start=False, stop=(kt == 1))
                return ps

            def gates_block(xt, S, hs=None, first_split=False):
                """iou gates for S nodes -> (it, ut, ot) [P, 2, S] fp16.
                Gate tiles are shared across phases (bufs=1): the DVE combine
                consumes them long before the ACT queue wraps around.
                For S<=1024, two gate psums are carved from each 4-bank tile
                so the PE gets twice the runway ahead of the ACT drain.
                first_split halves the very first activation call so the ACT
                engine starts as soon as half the psum is filled (ramp)."""
                tiles = {}
                idx, ps_tile = 0, [None]
                for gi, fn, nm in GATES:
                    gt = work.tile([P, 2, 2048], F16, tag="g" + nm,
                                   bufs=2 if nm == "o" else 1, name="g" + nm)
                    for ft in range(2):
                        bias = bt[:, gi * 2 + ft:gi * 2 + ft + 1]
                        if S <= 1024:
                            if idx % 2 == 0:
                                ps_tile[0] = psum.tile([P, 2048], F32,
                                                       tag="PS", bufs=2,
                                                       name="ps")
                            ps = ps_tile[0][:, (idx % 2) * S:(idx % 2 + 1) * S]
                        else:
                            ps = psum.tile([P, 2048], F32, tag="PS", bufs=2,
                                           name="ps")[:, :S]
                        if first_split and idx == 0:
                            half = S // 2
                            fill_iou(ps[:, :half], xt[:, :, :half], half,
                                     gi, ft)
                            nc.scalar.activation(gt[:, ft, :half],
                                                 ps[:, :half], fn, bias=bias)
                            fill_iou(ps[:, half:], xt[:, :, half:], half,
                                     gi, ft)
                            nc.scalar.activation(gt[:, ft, half:S],
                                                 ps[:, half:], fn, bias=bias)
                        else:
                            fill_iou(ps, xt, S, gi, ft, hs)
                            nc.scalar.activation(gt[:, ft, :S], ps, fn,
                                                 bias=bias)
                        idx += 1
                    tiles[nm] = gt[:, :, :S]
                return tiles["i"], tiles["u"], tiles["o"]

            # tanh(x) ~ x*(TA + TB*x^2 + TC*x^4), |x|<1, max err 3.9e-4
            TA, TB, TC = 0.99716201194203, -0.30798057777778, 0.07279929018615

            def combine(it, ut, ot, c_dst, h_dst, fcs=None, poly=False):
                """c = i*u (+ fcs); h = o*tanh(c). tanh reuses ut storage,
                or (poly=True, leaf-only: |c|<1) runs as a degree-5 odd
                polynomial on the Vector engine to off-load the ACT engine."""
                with nc.allow_low_precision(reason="fp16 by design"):
                    nc.vector.tensor_tensor(c_dst, it, ut, AluOpType.mult)
                    if fcs is not None:
                        nc.vector.tensor_tensor(c_dst, c_dst, fcs, AluOpType.add)
                    if poly:
                        s1 = work.tile([P, 2, 2048], F16, tag="pol1", bufs=1,
                                       name="pol1")[:, :, :c_dst.shape[-1]]
                        s2 = work.tile([P, 2, 2048], F16, tag="pol2", bufs=1,
                                       name="pol2")[:, :, :c_dst.shape[-1]]
                        nc.vector.tensor_tensor(s1, c_dst, c_dst,
                                                AluOpType.mult)
                        nc.vector.tensor_scalar(s2, s1, TC, TB,
                                                AluOpType.mult, AluOpType.add)
                        nc.vector.tensor_tensor(s2, s2, s1, AluOpType.mult)
                        nc.vector.scalar_tensor_tensor(s2, s2, TA, c_dst,
                                                       AluOpType.add,
                                                       AluOpType.mult)
                        nc.vector.tensor_tensor(h_dst, ot, s2, AluOpType.mult)
                    else:
                        nc.scalar.activation(ut, c_dst, AFT.Tanh)
                        nc.vector.tensor_tensor(h_dst, ot, ut, AluOpType.mult)

            def emit_hsum(ch_h, dst, Sp):
                """Sum 4-child groups of ch_h [P,2,4*Sp] into dst [P,2,Sp]."""
                with nc.allow_low_precision(reason="fp16 by design"):
                    htmp = work.tile([P, 2, 512, 2], F16, tag="htmp", bufs=2,
                                     name="htmp")
                    for ft in range(2):
                        v = ch_h[:, ft, :].rearrange("p (n b) -> p n b", b=BR)
                        nc.gpsimd.tensor_add(htmp[:, ft, :Sp, :],
                                             v[:, :, 0:2], v[:, :, 2:4])
                        nc.gpsimd.tensor_add(dst[:, ft, :],
                                             htmp[:, ft, :Sp, 0],
                                             htmp[:, ft, :Sp, 1])

            def emit_fprod(f_sb, ch_c, dst, Sp):
                """dst[P,2,Sp] = sum_children sigmoid(f) * child_c."""
                with nc.allow_low_precision(reason="fp16 by design"):
                    nc.vector.tensor_tensor(f_sb, f_sb, ch_c, AluOpType.mult)
                    for ft in range(2):
                        nc.vector.tensor_reduce(
                            dst[:, ft, :],
                            f_sb[:, ft, :].rearrange("p (n b) -> p n b", b=BR),
                            mybir.AxisListType.X, AluOpType.add)

            # ---------------- leaf phase, pipelined with level-7 f ----------
            leaf_hc = [None] * NG

            def emit_leaf(g):
                if g + 1 < NG:
                    o, w = GROUPS[g + 1]
                    t = xin.tile([P, 2, 2048], F16, tag="xleaf", bufs=2,
                                 name="xleaf")
                    nc.sync.dma_start(
                        t[:, :, :w],
                        xT[:, :, LOFF[8] + o:LOFF[8] + o + w]
                        .rearrange("a p n -> p a n"))
                    xt_g.append(t)
                o, w = GROUPS[g]
                xt = xt_g[g]
                it, ut, ot = gates_block(xt[:, :, :w], w, first_split=(g == 0))
                h8 = leafg.tile([P, 2, 2048], F16, tag="h8", bufs=2, name="h8")
                c8 = leafg.tile([P, 2, 2048], F16, tag="c8", bufs=2, name="c8")
                combine(it, ut, ot, c8[:, :, :w], h8[:, :, :w],
                        poly=(g < NG - 1))
                emit_hsum(h8[:, :, :w], hs7[:, :, o // 4:(o + w) // 4], w // 4)
                leaf_hc[g] = (h8, c8)

            def emit_f7(g):
                o, w = GROUPS[g]
                h8, c8 = leaf_hc[g]
                f_sb = work.tile([P, 2, 2048], F16, tag="f7", bufs=2,
                                 name="f7")
                for ft in range(2):
                    ps = f_psum(h8[:, :, :w], x7[:, :, o // 4:(o + w) // 4],
                                w // 4, ft)
                    nc.scalar.activation(f_sb[:, ft, :w], ps[:, :w],
                                         AFT.Sigmoid, bias=bt[:, 6 + ft:7 + ft])
                emit_fprod(f_sb[:, :, :w], c8[:, :, :w],
                           fcs7[:, :, o // 4:(o + w) // 4], w // 4)

            # ---------------- level 7 iou waves, outputs streamed out ------
            def emit_iou7(p0, p1, sub=1):
                S = p1 - p0
                s = slice(p0, p1)
                it, ut, ot = gates_block(x7[:, :, s], S, hs=hs7[:, :, s])
                w = S // sub
                with nc.allow_low_precision(reason="fp16 by design"):
                    for j in range(sub):
                        t = slice(p0 + j * w, p0 + (j + 1) * w)
                        jw = slice(j * w, (j + 1) * w)
                        # c first, DMA'd out while tanh/h still compute
                        nc.vector.tensor_tensor(c7[:, :, t], it[:, :, jw],
                                                ut[:, :, jw], AluOpType.mult)
                        nc.vector.tensor_tensor(c7[:, :, t], c7[:, :, t],
                                                fcs7[:, :, t], AluOpType.add)
                        nc.sync.dma_start(
                            out_c[:, :, t].rearrange("a p n -> p a n"),
                            c7[:, :, t])
                        nc.scalar.activation(ut[:, :, jw], c7[:, :, t],
                                             AFT.Tanh)
                        nc.vector.tensor_tensor(h7[:, :, t], ot[:, :, jw],
                                                ut[:, :, jw], AluOpType.mult)
                        nc.sync.dma_start(
                            out_h[:, :, t].rearrange("a p n -> p a n"),
                            h7[:, :, t])

            emit_leaf(0)
            emit_leaf(1)
            emit_f7(0)
            emit_leaf(2)
            emit_f7(1)
            emit_leaf(3)
            emit_f7(2)
            emit_iou7(0, 1024)
            emit_f7(3)
            emit_iou7(1024, 2048, sub=2)

    nc.compile()
    return nc


def level_offs():
    return [(BR ** l - 1) // (BR - 1) for l in range(D + 1)]


def shard_inputs(x, W_iou_x, b_iou_x, W_iou_h, b_iou_h, W_fx, b_fx, W_fh, b_fh,
                 *_ignored):
    offs = level_offs()
    wx_cat = np.concatenate([W_iou_x, W_fx], axis=0)
    wh_cat = np.concatenate([W_iou_h, W_fh], axis=0)
    wx_d = np.ascontiguousarray(wx_cat.T).reshape(2, P, 1024).astype(np.float16)
    wh_d = np.ascontiguousarray(wh_cat.T).reshape(2, P, 1024).astype(np.float16)
    b_iou = (b_iou_x + b_iou_h).reshape(6, P).T
    b_f = (b_fx + b_fh).reshape(2, P).T
    bias = np.ascontiguousarray(
        np.concatenate([b_iou, b_f], axis=1)).astype(np.float32)
    in_maps = []
    for k in range(NCORES):
        rows = []
        for l in range(CUT, D + 1):
            n = NLOC[l]
            rows.append(x[offs[l] + k * n: offs[l] + (k + 1) * n])
        xl = np.concatenate(rows, axis=0)
        xTk = np.ascontiguousarray(xl.T).reshape(2, P, -1).astype(np.float16)
        in_maps.append({"xT": xTk, "wx": wx_d, "wh": wh_d, "bias": bias})
    return in_maps


def finish_host(results, x, W_iou_x, b_iou_x, W_iou_h, b_iou_h,
                W_fx, b_fx, W_fh, b_fh, *_ignored):
    ncut = BR ** CUT
    npc = ncut // NCORES
    Hc = np.empty((ncut, 256), np.float32)
    Cc = np.empty((ncut, 256), np.float32)
    for k in range(NCORES):
        oh = results[k]["out_h"].astype(np.float32).reshape(256, npc)
        oc = results[k]["out_c"].astype(np.float32).reshape(256, npc)
        Hc[k * npc:(k + 1) * npc] = oh.T
        Cc[k * npc:(k + 1) * npc] = oc.T
    sig = lambda v: 1.0 / (1.0 + np.exp(-v))
    h_next, c_next = Hc, Cc
    for l in range(CUT - 1, -1, -1):
        n, off = BR ** l, (BR ** l - 1) // 3
        xl = x[off:off + n]
        child_h = h_next.reshape(n, BR, 256)
        child_c = c_next.reshape(n, BR, 256)
        chs = child_h.sum(axis=1)
        iou = xl @ W_iou_x.T + b_iou_x + chs @ W_iou_h.T + b_iou_h
        i, o, u = np.split(iou, 3, axis=1)
        i, o, u = sig(i), sig(o), np.tanh(u)
        f = sig(child_h @ W_fh.T + b_fh + (xl @ W_fx.T + b_fx)[:, None, :])
        c = i * u + (f * child_c).sum(axis=1)
        h = o * np.tanh(c)
        h_next, c_next = h, c
    return c_next.astype(np.float32), h_next.astype(np.float32)


# ---------------- public API ----------------

_D = D
_CUT = CUT
_CACHE = {}


def _get_program():
    if "nc" not in _CACHE:
        _CACHE["nc"] = build_program()
    return _CACHE["nc"]


def kernel(x, W_iou_x, b_iou_x, W_iou_h, b_iou_h, W_fx, b_fx, W_fh, b_fh):
    from concourse import bass_utils
    x = np.asarray(x, dtype=np.float32)
    args = [np.asarray(a, dtype=np.float32) for a in
            (W_iou_x, b_iou_x, W_iou_h, b_iou_h, W_fx, b_fx, W_fh, b_fh)]
    nc = _get_program()
    in_maps = shard_inputs(x, *args)
    res = bass_utils.run_bass_kernel_spmd(nc, in_maps,
                                          core_ids=list(range(NCORES)))
    c, h = finish_host(res.results, x, *args)
    return c, h
